# revision 3
# baseline (speedup 1.0000x reference)
"""Causal self-attention (B=2, T=2048, C=1024, H=16, Dh=64) on 8 TRN2 NeuronCores.

Sharding: tensor-parallel over heads — core c owns heads (2c, 2c+1) for both
batch elements; row-parallel output projection; host sums the 8 bf16 partials.

Structure (tile scheduler = priority list scheduler; emission order sets
priority, data deps set legality — every tile read must FOLLOW its write in
program order):
  - DMAs are batched 3D-AP transfers: 4 x-chunks per batch, one output DMA
    per 128-token block — ~46 DMAs total (HWDGE fixed cost is 625ns/DMA;
    the old per-piece scheme burned 84us of HWDGE on 135 DMAs).
  - attention chunk epilogues (1/r, broadcast, o-mul, projection, evict,
    DMA) are deferred and emitted as filler pieces inside the NEXT chunk's
    jt loop — their priority lands where their deps resolve.
  - attV lags scores by one jt in emission order.
  - V(b0) is emitted interleaved with K(b0) during the x-DMA window;
    V(b1) feeds progressively through a feeder drained just before each
    attV that needs it (attV(jt) only needs V tile tt=jt).
  - normalization: 1/r via DVE InstReciprocal straight from PSUM row 64
    (softmax denominator rides the attV accumulation as a ones-column),
    bf16-rounded only (no hi/lo split; measured 4.5e-3 rel err on HW),
    broadcast across partitions by two K=1 ones-matmuls into one PSUM
    tile; ScalarE runs only exp + the QK bias-add evictions (Identity
    with per-partition bias, same ACT table set as Exp).
  - projection is o-stationary: output tiles are token-major [128, 1024]
    bf16, host sums 8 partials without transposing. GPSIMD (Pool) cannot
    touch PSUM, so PSUM evictions live on DVE; Pool handles the causal
    mask multiplies and SBUF-to-SBUF rhi downcasts.
  - engine busy (TimelineSim): PE 118us, ACT 85us, DVE 77us, Pool 36us,
    DMA 52us; total 170us vs 185us for the previous kernel.
"""

import sys

sys.path.insert(0, "/opt/trn_rl_repo")

import numpy as np

D_MODEL = 1024
N_HEADS = 16
HEAD_DIM = 64
B = 2
T = 2048
N_CORES = 8
HPC = N_HEADS // N_CORES  # heads per core = 2
SCALE = 1.0 / np.sqrt(HEAD_DIM).astype(np.float32)

_STATE: dict = {}


def _patch_act_tables():
    """Pin Exp to the natural_log_exp_and_others table set so the kernel
    needs exactly one ACT table load."""
    import concourse.bacc as bacc_mod
    from concourse import mybir

    if getattr(bacc_mod, "_act_tables_patched", False):
        return
    FT = mybir.ActivationFunctionType
    orig = bacc_mod.get_activation_tables

    def patched(arch):
        tabs = orig(arch)
        out = {}
        for name, fns in tabs.items():
            if name != "natural_log_exp_and_others":
                fns = fns - {FT.Exp, FT.Ln}
            out[name] = fns
        return out

    bacc_mod.get_activation_tables = patched
    bacc_mod._act_tables_patched = True


def _build(reps=1):
    import concourse.bass as bass
    import concourse.tile as tile
    from concourse import mybir, bacc

    _patch_act_tables()

    f32 = mybir.dt.float32
    bf16 = mybir.dt.bfloat16
    FT = mybir.ActivationFunctionType

    nc = bacc.Bacc(trn_type="TRN2", target_bir_lowering=False, debug=False,
                   num_devices=N_CORES)

    xt = nc.dram_tensor("xt", [B, 8, 128, T], bf16, kind="ExternalInput").ap()
    wqk = nc.dram_tensor("wqk", [128, 8, 256], bf16, kind="ExternalInput").ap()
    wv = nc.dram_tensor("wv", [128, 8, 130], bf16, kind="ExternalInput").ap()
    wp = nc.dram_tensor("wp", [128, D_MODEL], bf16, kind="ExternalInput").ap()
    bqk = nc.dram_tensor("bqk", [128, 2], f32, kind="ExternalInput").ap()
    bv = nc.dram_tensor("bv", [128, 130], f32, kind="ExternalInput").ap()
    maskt = nc.dram_tensor("maskt", [128, 128], bf16, kind="ExternalInput").ap()
    sel = nc.dram_tensor("sel", [128, 128], bf16, kind="ExternalInput").ap()
    yt = nc.dram_tensor("yt", [B, T, D_MODEL], bf16, kind="ExternalOutput").ap()

    NCH = T // 512  # 512-wide token chunks per batch = 4
    NTT = T // 128  # 128-wide token tiles per batch = 16

    with tile.TileContext(nc) as tc:
        with tc.tile_pool(name="consts", bufs=1) as consts, \
             tc.tile_pool(name="xts", bufs=2) as xts_pool, \
             tc.tile_pool(name="qk", bufs=6) as qk_pool, \
             tc.tile_pool(name="vx", bufs=2) as vx_pool, \
             tc.tile_pool(name="ee", bufs=14) as e_pool, \
             tc.tile_pool(name="oo", bufs=2) as o_pool, \
             tc.tile_pool(name="rr", bufs=8) as r_pool, \
             tc.tile_pool(name="bc", bufs=4) as bc_pool, \
             tc.tile_pool(name="ost", bufs=6) as out_pool, \
             tc.tile_pool(name="psa", bufs=2, space="PSUM") as psa_pool, \
             tc.tile_pool(name="pss", bufs=2, space="PSUM") as pss_pool, \
             tc.tile_pool(name="pso", bufs=2, space="PSUM") as pso_pool:

            wqk_sb = consts.tile([128, 8, 256], bf16)
            # K weights first (K matmuls lead), Q half rides behind x(b0)
            nc.sync.dma_start(wqk_sb[:, :, 128:256], wqk[:, :, 128:256])
            wv_sb = consts.tile([128, 8, 130], bf16)
            wp_sb = consts.tile([128, D_MODEL], bf16)
            bqk_sb = consts.tile([128, 2], f32)
            bv_sb = consts.tile([128, 130], f32)
            mask_sb = consts.tile([128, 128], bf16)
            sel_sb = consts.tile([128, 128], bf16)

            def emit_consts():
                nc.sync.dma_start(wv_sb[:], wv)
                nc.sync.dma_start(bqk_sb[:], bqk)
                nc.sync.dma_start(bv_sb[:], bv)
                nc.sync.dma_start(mask_sb[:], maskt)
                nc.sync.dma_start(sel_sb[:], sel)
                nc.sync.dma_start(wp_sb[:], wp)

            xts = {}   # b -> x tile [128, 8, T]
            qks = {}   # b -> [Q^T, K^T]
            vs = {}    # b -> V_ext
            os_ = {}   # b -> O^T

            def emit_xt_chunk(b, ch):
                if b not in xts:
                    xts[b] = xts_pool.tile([128, 8, T], bf16, name=f"xt_{b}",
                                           tag="xt")
                nc.sync.dma_start(
                    xts[b][:, :, ch * 512:(ch + 1) * 512],
                    xt[b, :, :, ch * 512:(ch + 1) * 512].rearrange(
                        "c p t -> p c t"))

            def emit_qk_chunk(b, jq, ch):
                # jq: 0 = Q, 1 = K; one 512-token chunk
                if b not in qks:
                    qks[b] = [
                        qk_pool.tile([128, T], bf16, name=f"qk_{b}_{j}", tag="qk")
                        for j in range(2)]
                dst = qks[b][jq]
                ps = psa_pool.tile([128, 512], f32, name=f"pqk_{b}_{jq}_{ch}",
                                   tag="acc")
                for ct in range(8):
                    nc.tensor.matmul(
                        ps[:], wqk_sb[:, ct, jq * 128:(jq + 1) * 128],
                        xts[b][:, ct, ch * 512:(ch + 1) * 512],
                        start=(ct == 0), stop=(ct == 7))
                nc.scalar.activation(
                    dst[:, ch * 512:(ch + 1) * 512], ps[:], FT.Identity,
                    bias=bqk_sb[:, jq:jq + 1])

            def get_v_tile(b):
                if b not in vs:
                    vs[b] = vx_pool.tile([128, NTT * 130], bf16, name=f"v_{b}",
                                         tag="v")
                return vs[b]

            def emit_v(b, tt0, tt1):
                v_sb = get_v_tile(b)
                for tt in range(tt0, tt1):
                    psv = psa_pool.tile([128, 130], f32, name=f"pv_{b}_{tt}",
                                        tag="acc")
                    for ct in range(8):
                        nc.tensor.matmul(
                            psv[:], xts[b][:, ct, tt * 128:(tt + 1) * 128],
                            wv_sb[:, ct, :],
                            start=(ct == 0), stop=(ct == 7))
                    nc.vector.tensor_add(
                        v_sb[:, tt * 130: tt * 130 + 130], psv[:, 0:130], bv_sb[:])

            def emit_attn_core(b, ic, fillers, v_feeder=None):
                """Scores/exp/attV for chunk (b, ic); attV lags scores by one
                jt; one filler piece emitted per jt slot (from jt>=2).
                v_feeder: list of (coverage, piece) producing V tiles; drained
                until coverage > jt before emit_attv(jt).
                Ends with the 1/r DVE chain; returns state for epi_pieces."""
                if b not in os_:
                    os_[b] = o_pool.tile([128, T], bf16, name=f"o_{b}", tag="ot")
                q_sb, k_sb = qks[b][0], qks[b][1]
                v_sb = get_v_tile(b)
                psos = [pso_pool.tile([65, 512], f32, name=f"pso_{b}_{ic}_{h}",
                                      tag="o") for h in range(HPC)]
                njt = 4 * ic + 4
                ets = [None] * njt
                geom = []
                for jt in range(njt):
                    i_lo = max(512 * ic, 128 * jt)
                    geom.append((i_lo, 512 * (ic + 1) - i_lo))

                def emit_scores(jt):
                    i_lo, n_i = geom[jt]
                    ps2 = pss_pool.tile([128, 1024], f32,
                                        name=f"ps_{b}_{ic}_{jt}", tag="s")
                    for h in range(HPC):
                        hp = slice(h * 64, (h + 1) * 64)
                        nc.tensor.matmul(
                            ps2[:, h * 512: h * 512 + n_i],
                            k_sb[hp, jt * 128:(jt + 1) * 128],
                            q_sb[hp, i_lo:i_lo + n_i],
                            start=True, stop=True)
                    et = e_pool.tile([128, 1024], bf16, name=f"e_{b}_{ic}_{jt}",
                                     tag="e")
                    ets[jt] = et
                    if n_i == 512:
                        nc.scalar.activation(et[:], ps2[:], FT.Exp)
                    else:
                        nc.scalar.activation(
                            et[:].rearrange("p (h n) -> p h n", h=2)[:, :, 0:n_i],
                            ps2[:].rearrange("p (h n) -> p h n", h=2)[:, :, 0:n_i],
                            FT.Exp)
                    if jt >= 4 * ic:
                        for h in range(HPC):
                            nc.gpsimd.tensor_mul(
                                et[:, h * 512: h * 512 + 128],
                                et[:, h * 512: h * 512 + 128], mask_sb[:])

                def emit_attv(jt):
                    if v_feeder is not None:
                        while v_feeder and v_feeder[0][0] <= jt:
                            v_feeder.pop(0)[1]()
                    i_lo, n_i = geom[jt]
                    for h in range(HPC):
                        nc.tensor.matmul(
                            psos[h][:, i_lo - 512 * ic: 512],
                            v_sb[:, jt * 130 + h * 65: jt * 130 + (h + 1) * 65],
                            ets[jt][:, h * 512: h * 512 + n_i],
                            start=(jt == 0), stop=(jt == njt - 1))

                emit_scores(0)
                for jt in range(1, njt):
                    emit_scores(jt)
                    if jt >= 2 and fillers:
                        fillers.popleft()[1]()
                    emit_attv(jt - 1)
                emit_attv(njt - 1)
                if v_feeder is not None:
                    while v_feeder:
                        v_feeder.pop(0)[1]()
                # 1/r on DVE straight from PSUM row 64 (per-head tiles —
                # engine APs must start at a quarter-aligned partition).
                rhis = []
                for h in range(HPC):
                    r_t = r_pool.tile([65, 512], f32, name=f"r_{b}_{ic}_{h}",
                                      tag="r")
                    rhi = r_pool.tile([65, 512], bf16, name=f"rhi_{b}_{ic}_{h}",
                                      tag="rhi")
                    nc.vector.reciprocal(r_t[64:65, :], psos[h][64:65, :])
                    nc.gpsimd.tensor_copy(rhi[64:65, :], r_t[64:65, :])
                    rhis.append(rhi)
                return psos, rhis

            def epi_pieces(b, ic, psos, rhis):
                """Filler-piece callables finishing chunk (b, ic):
                broadcast + o-mul, then 8 projection half-pieces (matmul +
                eviction; output DMA rides on the odd halves)."""
                o_sb = os_[b]

                def bc_piece():
                    bc_ps = psa_pool.tile([128, 512], f32, name=f"bcp_{b}_{ic}",
                                          tag="acc")
                    for h in range(HPC):
                        nc.tensor.matmul(
                            bc_ps[h * 64:(h + 1) * 64, :],
                            sel_sb[64:65, 0:64], rhis[h][64:65, :],
                            start=True, stop=True)
                    bc_sb = bc_pool.tile([128, 512], f32, name=f"bcs_{b}_{ic}",
                                         tag="bc")
                    nc.vector.tensor_copy(bc_sb[:], bc_ps[:])
                    for h in range(HPC):
                        hp = slice(h * 64, (h + 1) * 64)
                        nc.vector.tensor_mul(
                            o_sb[hp, ic * 512:(ic + 1) * 512], psos[h][0:64, :],
                            bc_sb[h * 64:(h + 1) * 64, :])

                osts = {}

                def proj_piece(ib, half):
                    i0 = ic * 512 + ib * 128
                    if ib not in osts:
                        osts[ib] = out_pool.tile(
                            [128, D_MODEL], bf16,
                            name=f"ost_{b}_{ic}_{ib}", tag="ost")
                    ost = osts[ib]
                    pp = psa_pool.tile([128, 512], f32,
                                       name=f"pp_{b}_{ic}_{ib}_{half}",
                                       tag="acc")
                    nc.tensor.matmul(
                        pp[:], o_sb[:, i0:i0 + 128],
                        wp_sb[:, half * 512:(half + 1) * 512],
                        start=True, stop=True)
                    nc.vector.tensor_copy(
                        ost[:, half * 512:(half + 1) * 512], pp[:])
                    if half == 1:
                        nc.sync.dma_start(yt[b, i0:i0 + 128, :], ost[:])

                yield bc_piece
                for ib in range(4):
                    for half in range(2):
                        yield (lambda ib=ib, half=half: proj_piece(ib, half))

            # ---- emission schedule ----
            from collections import deque

            for rep in range(reps):
                xts.clear(); qks.clear(); vs.clear(); os_.clear()
                emit_xt_chunk(0, 0)
                emit_xt_chunk(0, 1)
                if rep == 0:
                    nc.sync.dma_start(wqk_sb[:, :, 0:128], wqk[:, :, 0:128])
                    nc.sync.dma_start(bqk_sb[:], bqk)
                    nc.sync.dma_start(wv_sb[:], wv)
                    nc.sync.dma_start(bv_sb[:], bv)
                    nc.sync.dma_start(mask_sb[:], maskt)
                    nc.sync.dma_start(sel_sb[:], sel)
                emit_xt_chunk(0, 2)
                emit_xt_chunk(0, 3)
                if rep == 0:
                    nc.sync.dma_start(wp_sb[:], wp)
                for ch in range(4):
                    emit_xt_chunk(1, ch)

                # fillers: (is_producer, fn). Producers write tiles consumed
                # by later cores and must be force-emitted before them; sinks
                # (epilogues) are safe to emit any time after their chunk.
                fillers = deque()

                def pull():
                    if fillers:
                        fillers.popleft()[1]()

                def flush(producers_only=False):
                    if producers_only:
                        keep = deque()
                        while fillers:
                            isp, fn = fillers.popleft()
                            if isp:
                                fn()
                            else:
                                keep.append((isp, fn))
                        fillers.extend(keep)
                    else:
                        while fillers:
                            fillers.popleft()[1]()

                def run_core(b, ic, v_feeder=None):
                    st = emit_attn_core(b, ic, fillers, v_feeder)
                    fillers.extend(
                        (False, p) for p in epi_pieces(b, ic, *st))

                # K(b0) + V(b0) interleaved during the x-DMA window,
                # then Q(b0, ch3) -> chunk (0,3) can start.
                for ch in range(4):
                    emit_qk_chunk(0, 1, ch)
                    emit_v(0, 4 * ch, 4 * ch + 2)
                    emit_v(0, 4 * ch + 2, 4 * ch + 4)
                emit_qk_chunk(0, 0, 3)
                # remaining Q(b0) + all K(b1) + Q(b1, ch3) ride as fillers
                fillers.append((True, lambda: emit_qk_chunk(0, 0, 2)))
                fillers.append((True, lambda: emit_qk_chunk(0, 0, 1)))
                fillers.append((True, lambda: emit_qk_chunk(0, 0, 0)))
                for ch in range(4):
                    fillers.append((True, lambda c=ch: emit_qk_chunk(1, 1, c)))
                fillers.append((True, lambda: emit_qk_chunk(1, 0, 3)))
                run_core(0, 3)
                run_core(0, 2)
                run_core(0, 1)
                run_core(0, 0)
                # force not-yet-emitted K(b1)/Q(b1,3) producers out before
                # b1 attention consumes them; epilogue sinks stay queued
                flush(producers_only=True)
                feeder = [(t, (lambda t0=t: emit_v(1, t0, t0 + 2)))
                          for t in range(0, 16, 2)]
                run_core(1, 3, feeder)
                emit_qk_chunk(1, 0, 2)
                run_core(1, 2)
                emit_qk_chunk(1, 0, 1)
                run_core(1, 1)
                emit_qk_chunk(1, 0, 0)
                run_core(1, 0)
                flush()

    nc.finalize()
    return nc


def _prep_inputs(x, w_qkv, b_qkv, w_proj):
    """Host-side sharding/layout prep. Returns per-core in_maps."""
    import ml_dtypes

    x = np.asarray(x, dtype=np.float32)
    w_qkv = np.asarray(w_qkv, dtype=np.float32)
    b_qkv = np.asarray(b_qkv, dtype=np.float32)
    w_proj = np.asarray(w_proj, dtype=np.float32)

    wq = w_qkv[:, 0:D_MODEL].reshape(D_MODEL, N_HEADS, HEAD_DIM)
    wk = w_qkv[:, D_MODEL:2 * D_MODEL].reshape(D_MODEL, N_HEADS, HEAD_DIM)
    wv = w_qkv[:, 2 * D_MODEL:3 * D_MODEL].reshape(D_MODEL, N_HEADS, HEAD_DIM)
    bq = b_qkv[0:D_MODEL].reshape(N_HEADS, HEAD_DIM)
    bk = b_qkv[D_MODEL:2 * D_MODEL].reshape(N_HEADS, HEAD_DIM)
    bvv = b_qkv[2 * D_MODEL:3 * D_MODEL].reshape(N_HEADS, HEAD_DIM)

    xt = np.ascontiguousarray(
        x.transpose(0, 2, 1).reshape(B, 8, 128, T)).astype(ml_dtypes.bfloat16)
    mask = np.ascontiguousarray(
        (np.arange(128)[:, None] <= np.arange(128)[None, :])).astype(
            ml_dtypes.bfloat16)
    sel = np.zeros((128, 128), np.float32)
    sel[64, 0:64] = 1.0
    sel[65, 64:128] = 1.0
    sel = sel.astype(ml_dtypes.bfloat16)

    in_maps = []
    for c in range(N_CORES):
        h0, h1 = HPC * c, HPC * c + 1
        wqk_c = np.concatenate(
            [wq[:, h0] * SCALE, wq[:, h1] * SCALE, wk[:, h0], wk[:, h1]], axis=1)
        wv_c = np.zeros((D_MODEL, 130), np.float32)
        wv_c[:, 0:64] = wv[:, h0]
        wv_c[:, 65:129] = wv[:, h1]
        bqk_c = np.stack(
            [np.concatenate([bq[h0], bq[h1]]) * SCALE,
             np.concatenate([bk[h0], bk[h1]])], axis=1)  # [128, 2]
        bv_c = np.zeros((128, 130), np.float32)
        bv_c[:, 0:64] = bvv[h0][None, :]
        bv_c[:, 64] = 1.0
        bv_c[:, 65:129] = bvv[h1][None, :]
        bv_c[:, 129] = 1.0
        wp_c = w_proj[128 * c:128 * (c + 1), :]
        in_maps.append({
            "xt": xt,
            "wqk": np.ascontiguousarray(
                wqk_c.reshape(8, 128, 256).transpose(1, 0, 2)).astype(
                    ml_dtypes.bfloat16),
            "wv": np.ascontiguousarray(
                wv_c.reshape(8, 128, 130).transpose(1, 0, 2)).astype(
                    ml_dtypes.bfloat16),
            "wp": np.ascontiguousarray(wp_c).astype(ml_dtypes.bfloat16),
            "bqk": np.ascontiguousarray(bqk_c),
            "bv": bv_c,
            "maskt": mask,
            "sel": sel,
        })
    return in_maps


def kernel(x, w_qkv, b_qkv, w_proj, b_proj):
    import os
    from concourse.bass_utils import run_bass_kernel_spmd

    if "nc" not in _STATE:
        _STATE["nc"] = _build()
    nc = _STATE["nc"]

    in_maps = _prep_inputs(x, w_qkv, b_qkv, w_proj)
    prev = os.environ.get("BASS_NEVER_TRACE")
    os.environ["BASS_NEVER_TRACE"] = "1"
    try:
        res = run_bass_kernel_spmd(nc, in_maps, core_ids=list(range(N_CORES)))
    finally:
        if prev is None:
            os.environ.pop("BASS_NEVER_TRACE", None)
        else:
            os.environ["BASS_NEVER_TRACE"] = prev

    acc = np.zeros((B, T, D_MODEL), np.float32)
    for c in range(N_CORES):
        acc += np.asarray(res.results[c]["yt"], np.float32)
    out = acc + np.asarray(b_proj, np.float32)[None, None, :]
    return np.ascontiguousarray(out)
